# revision 11
# baseline (speedup 1.0000x reference)
"""Trainium2 Bass kernel for nn_ModalDecoder (embedding_lookup).

Reference computation:
    w  = out_projection_table[idx].reshape(B, F, D, O)      # [B,F,D,O]
    b  = feature_bias_table[idx]                            # [B,F,D]
    xb = x[:, :, None, :] + b[:, None, :, :]                # [B,N,F,D]
    out = einsum('bnfd,bfdo->bnfo', xb, w)                  # [B,N,F,O]

Factorization (avoids the 128MB [B,N,F,D] intermediate):
    out[b, n, f, :] = x[b, n, :] @ W[b, f] + (bias[b, f] @ W[b, f])
The bias term is a per-(b,f) length-O vector, broadcast over n; it is
precomputed on host and added on-device during the PSUM->SBUF drain
(tensor_scalar_add with a per-partition scalar).

Sharding: 8 cores = 4 values of b x 2 halves of N. Per core:
    y[fo, n] = Wpack[d, fo].T @ xT[d, n] + cvec[fo]
with Wpack = [D, F*O] (host-gathered tables packed side by side), xT the
transposed x half, both bf16 (PSUM accumulates fp32). y is [F*O, N/2] fp16
(upcast on host; fp16 keeps ~2^-11 mantissa so bf16 matmul rounding still
dominates the error).

Schedule (v3), built around measured DMA physics (per-transfer end-to-end
sem latency ~2.5-3us = issue 0.65 + ring startup ~0.8 + stream + receipt
~1.0; ring FIFO completion): ALL loads ride the sync HWDGE ring in
PE-gating order with ONE cumulative semaphore (FIFO makes inc order
deterministic): xt-k0, wp-g0, xt-k1, xt-k23, wp-g1, wp-g23, wp-g45,
wp-g67. The first two transfers are small so the first real matmul can
start ~3.2us after block start. 8 warmup matmuls keep the PE busy from
block start until that gate, so the HAM clock-gate un-throttles (1.2 ->
2.4 GHz) ~4.5us in with no idle gap to reset the activity window. Drains
run on DVE; stores alternate rings per group (cv loads first on the
scalar ring, priming it). The last group is drained in halves (DVE +
ACT in parallel) and stored as two 64KB transfers on both rings to
minimize the post-last-matmul tail. No end-of-kernel completion waits:
the NEFF epilogue's DRAIN retires in-flight DMAs.

Per-core HBM traffic: 0.5MB xT + 1MB Wpack + 1MB out (memory-bound).
"""

import numpy as np
import ml_dtypes

B, N, D, O, F, V = 4, 1024, 512, 64, 16, 64
NH = N // 2            # 512 rows of x per core
FO = F * O             # 1024 packed output columns
KT = D // 128          # 4 contraction chunks
ST = FO // 128         # 8 output-partition chunks
SH = NH // 2           # half-group column split for the tail stores
N_WARM = 8             # PE warmup matmuls before the first load gate

_cache: dict = {}


def _build_program(with_clears=True):
    # with_clears=True is the real (HW) program. The False variant is for
    # CoreSim validation: it enables the race detector and memsets the
    # warmup scratch (CoreSim rejects reads of uninitialized SBUF; on HW
    # the warmup matmul inputs are garbage by design and never observed).
    import concourse.bass as bass
    import concourse.mybir as mybir

    bf16 = mybir.dt.bfloat16
    f16 = mybir.dt.float16
    f32 = mybir.dt.float32

    nc = bass.Bass(
        "TRN2",
        target_bir_lowering=False,
        debug=False,
        num_devices=8,
        detect_race_conditions=not with_clears,
    )

    xt_d = nc.dram_tensor("xt", [128, KT * NH], bf16, kind="ExternalInput")
    wp_d = nc.dram_tensor("wp", [128, KT * FO], bf16, kind="ExternalInput")
    cv_d = nc.dram_tensor("cv", [128, ST], f32, kind="ExternalInput")
    y_d = nc.dram_tensor("y", [FO, NH], f16, kind="ExternalOutput")

    yv = y_d.ap().rearrange("(g p) n -> p g n", p=128)  # [128, ST, NH]

    with (
        nc.sbuf_tensor("xt_sb", [128, KT * NH], bf16) as xt_sb,
        nc.sbuf_tensor("wp_sb", [128, KT * FO], bf16) as wp_sb,
        nc.sbuf_tensor("cv_sb", [128, ST], f32) as cv_sb,
        nc.sbuf_tensor("out_sb", [128, ST, NH], f16) as out_sb,
        nc.sbuf_tensor("scr_sb", [128, NH], bf16) as scr_sb,
        nc.psum_tensor([128, ST, NH], f32) as ps,
        nc.semaphore("s_ld1") as s_ld1,
        nc.semaphore("s_ld2") as s_ld2,
        nc.semaphore("s_ld3") as s_ld3,
        nc.semaphore("s_ld4") as s_ld4,
        nc.semaphore("s_ld5") as s_ld5,
        nc.semaphore("s_ld6") as s_ld6,
        nc.semaphore("s_ld7") as s_ld7,
        nc.semaphore("s_cv") as s_cv,
        nc.semaphore("s_ws") as s_ws,
        nc.semaphore("s_mm") as s_mm,
        nc.semaphore("s_dve_sync") as s_dve_sync,
        nc.semaphore("s_dve_act") as s_dve_act,
        nc.semaphore("s_st") as s_st,
        nc.Block() as block,
    ):

        @block.sync
        def _(sync):
            # All loads on one ring, in PE-gating order, one cumulative
            # semaphore (FIFO completion): after chunk j lands, s_ld ==
            # (j+1)*16. First two chunks are small to advance the first
            # matmul gate.
            sync.dma_start(xt_sb[:, 0:512], xt_d.ap()[:, 0:512]).then_inc(s_ld1, 16)
            sync.dma_start(wp_sb[:, 0:512], wp_d.ap()[:, 0:512]).then_inc(s_ld1, 16)
            sync.dma_start(xt_sb[:, 512:1024], xt_d.ap()[:, 512:1024]).then_inc(
                s_ld2, 16
            )
            sync.dma_start(xt_sb[:, 1024:2048], xt_d.ap()[:, 1024:2048]).then_inc(
                s_ld3, 16
            )
            sync.dma_start(wp_sb[:, 512:1024], wp_d.ap()[:, 512:1024]).then_inc(
                s_ld4, 16
            )
            sync.dma_start(wp_sb[:, 1024:2048], wp_d.ap()[:, 1024:2048]).then_inc(
                s_ld5, 16
            )
            sync.dma_start(wp_sb[:, 2048:3072], wp_d.ap()[:, 2048:3072]).then_inc(
                s_ld6, 16
            )
            sync.dma_start(wp_sb[:, 3072:4096], wp_d.ap()[:, 3072:4096]).then_inc(
                s_ld7, 16
            )
            # Even-group stores, then the first half of group 7.
            for j, s in enumerate((0, 2, 4, 6)):
                sync.wait_ge(s_dve_sync, j + 1)
                sync.dma_start(yv[:, s, :], out_sb[:, s, :]).then_inc(s_st, 16)
            sync.wait_ge(s_dve_sync, 5)
            sync.dma_start(yv[:, 7, 0:SH], out_sb[:, 7, 0:SH]).then_inc(s_st, 16)
            # No final completion wait: the framework epilogue's DRAIN
            # retires in-flight DMAs.

        @block.scalar
        def _(scalar):
            # cv primes this ring's DMA path; DVE needs it only at the
            # first drain (~+4.5us).
            scalar.dma_start(cv_sb[:], cv_d.ap()).then_inc(s_cv, 16)
            for j, s in enumerate((1, 3, 5)):
                scalar.wait_ge(s_dve_act, j + 1)
                scalar.dma_start(yv[:, s, :], out_sb[:, s, :]).then_inc(s_st, 16)
            # Tail: ACT drains the second half of group 7 itself (in
            # parallel with DVE's first half), then stores it.
            scalar.wait_ge(s_mm, ST)
            scalar.wait_ge(s_cv, 16)
            nc.scalar.add(
                out_sb[:, 7, SH:NH], ps[:, 7, SH:NH], cv_sb[:, 7:8]
            ).then_inc(s_dve_act, 1)
            scalar.wait_ge(s_dve_act, 4)
            scalar.dma_start(yv[:, 7, SH:NH], out_sb[:, 7, SH:NH]).then_inc(
                s_st, 16
            )

        @block.tensor
        def _(tensor):
            # Warm the PE HAM clock gate while the first loads are in
            # flight. scr_sb is never written on HW (garbage is fine -- the
            # warmup PSUM bank is overwritten with start=True by group ST-1
            # before any read); the sim variant memsets it first.
            if not with_clears:
                tensor.wait_ge(s_ws, 1)
            for _ in range(N_WARM):
                nc.tensor.matmul(
                    ps[:, ST - 1, :],
                    scr_sb[:, :128],
                    scr_sb[:],
                    start=True,
                    stop=True,
                )
            # Group-serial accumulation: group s finishes after its own 4
            # matmuls, so DVE drains + stores pipeline behind PE. wp columns
            # are laid out [s][k][fo_local]. Load-chunk gating via the
            # cumulative s_ld: chunk order xtk0, wp0, xtk1, xtk23, wp1,
            # wp23, wp45, wp67.
            for s in range(ST):
                if s == 0:
                    tensor.wait_ge(s_ld1, 32)   # xtk0 + wp0
                elif s == 1:
                    tensor.wait_ge(s_ld4, 16)   # wp1 (implies earlier chunks)
                elif s == 2:
                    tensor.wait_ge(s_ld5, 16)   # wp23
                elif s == 4:
                    tensor.wait_ge(s_ld6, 16)   # wp45
                elif s == 6:
                    tensor.wait_ge(s_ld7, 16)   # wp67
                for k in range(KT):
                    if s == 0 and k == 1:
                        tensor.wait_ge(s_ld2, 16)   # xtk1
                    elif s == 0 and k == 2:
                        tensor.wait_ge(s_ld3, 16)   # xtk23
                    inst = nc.tensor.matmul(
                        ps[:, s, :],
                        wp_sb[:, s * 512 + k * 128:s * 512 + (k + 1) * 128],
                        xt_sb[:, k * NH:(k + 1) * NH],
                        start=(k == 0),
                        stop=(k == KT - 1),
                    )
                    if k == KT - 1:
                        inst.then_inc(s_mm, 1)

        @block.vector
        def _(vector):
            if not with_clears:
                vector.memset(scr_sb[:], 0).then_inc(s_ws, 1)
            vector.wait_ge(s_cv, 16)  # cv loaded
            for s in range(ST - 1):
                vector.wait_ge(s_mm, s + 1)
                inst = nc.vector.tensor_scalar_add(
                    out_sb[:, s, :], ps[:, s, :], cv_sb[:, s:s + 1]
                )
                if s % 2 == 0:
                    inst.then_inc(s_dve_sync, 1)
                else:
                    inst.then_inc(s_dve_act, 1)
            # First half of group 7 (ACT covers the second half).
            vector.wait_ge(s_mm, ST)
            nc.vector.tensor_scalar_add(
                out_sb[:, 7, 0:SH], ps[:, 7, 0:SH], cv_sb[:, 7:8]
            ).then_inc(s_dve_sync, 1)

    return nc


def _get_program():
    nc = _cache.get("nc")
    if nc is None:
        nc = _build_program()
        _cache["nc"] = nc
    return nc


def _prep_in_maps(x, idx, fbt, opt):
    bf = ml_dtypes.bfloat16
    in_maps = []
    for b in range(B):
        w = opt[idx[b]].reshape(F, D, O)                     # [F,D,O] f32
        wpack = w.transpose(1, 0, 2).reshape(KT, 128, ST, 128)  # [k,p,s,c]
        wp_host = np.ascontiguousarray(
            wpack.transpose(1, 2, 0, 3).reshape(128, KT * FO)
        ).astype(bf)                                         # [p, s*512+k*128+c]
        bias = fbt[idx[b]]                                   # [F,D]
        cvec = np.einsum("fd,fdo->fo", bias, w).reshape(FO).astype(np.float32)
        cv = np.ascontiguousarray(cvec.reshape(ST, 128).T)   # [128, ST]
        for h in range(2):
            xtT = x[b, h * NH:(h + 1) * NH, :].T             # [D, NH]
            xt_host = np.ascontiguousarray(
                xtT.reshape(KT, 128, NH).transpose(1, 0, 2).reshape(128, KT * NH)
            ).astype(bf)                                     # [128, KT*NH]
            in_maps.append({"xt": xt_host, "wp": wp_host, "cv": cv})
    return in_maps


def _assemble(results):
    out = np.empty((B, N, F, O), dtype=np.float32)
    for c in range(8):
        b, h = divmod(c, 2)
        y = np.asarray(results[c]["y"]).astype(np.float32)   # [FO, NH]
        out[b, h * NH:(h + 1) * NH] = y.reshape(F, O, NH).transpose(2, 0, 1)
    return out


def _run(x, idx, feature_bias_table, out_projection_table, **run_kwargs):
    from concourse.bass_utils import run_bass_kernel_spmd

    x = np.asarray(x, dtype=np.float32)
    idx = np.asarray(idx).astype(np.int64)
    fbt = np.asarray(feature_bias_table, dtype=np.float32)
    opt = np.asarray(out_projection_table, dtype=np.float32)

    nc = _get_program()
    in_maps = _prep_in_maps(x, idx, fbt, opt)
    res = run_bass_kernel_spmd(nc, in_maps, core_ids=list(range(8)), **run_kwargs)
    return _assemble(res.results), res


def kernel(x, idx, feature_bias_table, out_projection_table):
    out, _ = _run(x, idx, feature_bias_table, out_projection_table)
    return out


# revision 20
# speedup vs baseline: 1.1342x; 1.1342x over previous
"""Trainium2 Bass kernel for nn_ModalDecoder (embedding_lookup).

Reference computation:
    w  = out_projection_table[idx].reshape(B, F, D, O)      # [B,F,D,O]
    b  = feature_bias_table[idx]                            # [B,F,D]
    xb = x[:, :, None, :] + b[:, None, :, :]                # [B,N,F,D]
    out = einsum('bnfd,bfdo->bnfo', xb, w)                  # [B,N,F,O]

Factorization (avoids the 128MB [B,N,F,D] intermediate):
    out[b, n, f, :] = x[b, n, :] @ W[b, f] + (bias[b, f] @ W[b, f])
The bias term is a per-(b,f) length-O vector, broadcast over n; it is
precomputed on host and added during the PSUM->SBUF drain (per-partition
scalar add on DVE / ACT).

Sharding: 8 cores = 4 values of b x 2 halves of N. Per core:
    y[fo, n] = Wpack[d, fo].T @ xT[d, n] + cvec[fo]
with Wpack = [D, F*O] (host-gathered tables packed side by side), xT the
transposed x half, both bf16 (PSUM accumulates fp32). y is [F*O, N/2] fp16
(upcast on host; the bf16 matmul rounding dominates the error).

Schedule (v4), tuned to measured DMA physics (per-dma issue ~0.65us of
sequencer time; ring startup ~0.8us; stream ~390GB/s; completion receipt
~1.0us after last byte; ring FIFO):
  - All loads ride the sync HWDGE ring from ONE fused DRAM tensor laid out
    in load order [xt01|wp0 | xt23 | wp1 | wp23 | wp45 | wp67], as 6
    transfers; the first (384KB) delivers everything matmul 1 needs with a
    single completion sem, and every later chunk lands with >=0.4us slack
    before the PE consumes it.
  - PE warmup runway: 6x512-free + 7x128-free dummy matmuls keep the PE
    busy from block start (HAM clock-gate window accumulates; warm
    2.4GHz ~4.5us in) and hand over to real matmuls with <=107ns
    granularity. Any PE idle before warm would reset the HAM window
    (costs ~4us - measured), so the runway overshoots the expected gate
    by ~0.4us on purpose.
  - Drains on DVE (749ns/group < 864ns PE group cadence), stores
    alternate rings per group. Tail: group 7 drains in halves, DVE || ACT
    (ACT's activation table is preloaded at block start - first use
    otherwise costs a 1.28us ACT_TABLE_LOAD), stored as two 64KB
    transfers on both rings.
  - HW-variant store DMAs carry no completion semaphore: nothing waits on
    them and the NEFF epilogue's queue DRAIN retires them; dropping the
    sem skips the last-byte-confirmed receipt in the measured span. The
    CoreSim variant keeps sems for the race detector.

Per-core HBM traffic: 1.5MB loads + 1MB stores (memory-bound).
"""

import numpy as np
import ml_dtypes

B, N, D, O, F, V = 4, 1024, 512, 64, 16, 64
NH = N // 2            # 512 rows of x per core
FO = F * O             # 1024 packed output columns
KT = D // 128          # 4 contraction chunks
ST = FO // 128         # 8 output-partition chunks
SH = NH // 2           # half-group column split for the tail stores
LD = KT * NH + KT * FO  # 6144 fused load columns

# Fused load-buffer column offsets (load order).
XT_COL = {0: 0, 1: 512, 2: 1536, 3: 2048}        # xt k-chunk -> col
WP_COL = {0: 1024, 1: 2560, 2: 3072, 3: 3584,
          4: 4096, 5: 4608, 6: 5120, 7: 5632}    # wp s-group -> col
# Load transfers: (col_start, col_end); chunk j gets its own semaphore.
LD_CHUNKS = [(0, 1536), (1536, 2560), (2560, 3072),
             (3072, 4096), (4096, 5120), (5120, 6144)]

_cache: dict = {}


def _build_program(with_clears=True):
    # with_clears=True is the real (HW) program. The False variant is for
    # CoreSim validation: it enables the race detector, memsets the warmup
    # scratch (CoreSim rejects reads of uninitialized SBUF; on HW the
    # warmup inputs are garbage by design and never observed), and adds
    # completion sems to the store DMAs (race-detector requirement; the HW
    # variant relies on the epilogue DRAIN instead).
    import concourse.bass as bass
    import concourse.mybir as mybir

    bf16 = mybir.dt.bfloat16
    f16 = mybir.dt.float16
    f32 = mybir.dt.float32

    nc = bass.Bass(
        "TRN2",
        target_bir_lowering=False,
        debug=False,
        num_devices=8,
        detect_race_conditions=not with_clears,
    )

    ld_d = nc.dram_tensor("ld", [128, LD], bf16, kind="ExternalInput")
    cv_d = nc.dram_tensor("cv", [128, ST], f32, kind="ExternalInput")
    y_d = nc.dram_tensor("y", [FO, NH], f16, kind="ExternalOutput")

    yv = y_d.ap().rearrange("(g p) n -> p g n", p=128)  # [128, ST, NH]

    with (
        nc.sbuf_tensor("ld_sb", [128, LD], bf16) as ld_sb,
        nc.sbuf_tensor("cv_sb", [128, ST], f32) as cv_sb,
        nc.sbuf_tensor("out_sb", [128, ST, NH], f16) as out_sb,
        nc.sbuf_tensor("scr_sb", [128, NH], bf16) as scr_sb,
        nc.psum_tensor([128, ST, NH], f32) as ps,
        nc.semaphore("s_l1") as s_l1,
        nc.semaphore("s_l2") as s_l2,
        nc.semaphore("s_l3") as s_l3,
        nc.semaphore("s_l4") as s_l4,
        nc.semaphore("s_l5") as s_l5,
        nc.semaphore("s_l6") as s_l6,
        nc.semaphore("s_cv") as s_cv,
        nc.semaphore("s_ws") as s_ws,
        nc.semaphore("s_mm") as s_mm,
        nc.semaphore("s_dve_sync") as s_dve_sync,
        nc.semaphore("s_dve_act") as s_dve_act,
        nc.semaphore("s_st") as s_st,
        nc.Block() as block,
    ):
        ld_sems = [s_l1, s_l2, s_l3, s_l4, s_l5, s_l6]

        def store(eng, dst, src):
            # HWDGE DMAs must carry a sem update (codegen requirement);
            # nothing waits on s_st -- the epilogue DRAIN handles retirement.
            eng.dma_start(dst, src).then_inc(s_st, 16)

        @block.sync
        def _(sync):
            # All loads on one ring, in PE-gating order; chunk j's sem
            # implies chunks <j landed (per-engine FIFO).
            for (c0, c1), sem in zip(LD_CHUNKS, ld_sems):
                sync.dma_start(ld_sb[:, c0:c1], ld_d.ap()[:, c0:c1]).then_inc(
                    sem, 16
                )
            # Even-group stores, then the first half of group 7.
            for j, s in enumerate((0, 2, 4, 6)):
                sync.wait_ge(s_dve_sync, j + 1)
                store(sync, yv[:, s, :], out_sb[:, s, :])
            sync.wait_ge(s_dve_sync, 5)
            store(sync, yv[:, 7, 0:SH], out_sb[:, 7, 0:SH])
            # No final completion wait: the framework epilogue's DRAIN
            # retires in-flight DMAs.

        @block.scalar
        def _(scalar):
            # cv primes this ring's DMA path; only DVE consumes it.
            # (No ACT compute anywhere: an activation instruction's
            # ACT_TABLE_LOAD DMA wedged the HW when issued concurrently
            # with this kernel's dynamic-DMA traffic.)
            scalar.dma_start(cv_sb[:], cv_d.ap()).then_inc(s_cv, 16)
            for j, s in enumerate((1, 3, 5)):
                scalar.wait_ge(s_dve_act, j + 1)
                store(scalar, yv[:, s, :], out_sb[:, s, :])
            # Tail: second half of group 7 (drained by DVE).
            scalar.wait_ge(s_dve_act, 4)
            store(scalar, yv[:, 7, SH:NH], out_sb[:, 7, SH:NH])

        @block.tensor
        def _(tensor):
            # Warmup runway: keeps the PE busy from block start until the
            # first load gate. Coarse then fine, so the handover to real
            # work wastes <=107ns. scr_sb is never written on HW.
            if not with_clears:
                tensor.wait_ge(s_ws, 1)
            for _ in range(6):
                nc.tensor.matmul(
                    ps[:, ST - 1, :],
                    scr_sb[:, :128],
                    scr_sb[:],
                    start=True,
                    stop=True,
                )
            for _ in range(7):
                nc.tensor.matmul(
                    ps[:, ST - 1, 0:128],
                    scr_sb[:, :128],
                    scr_sb[:, 0:128],
                    start=True,
                    stop=True,
                )
            # Group-serial accumulation: group s finishes after its own 4
            # matmuls, so DVE drains + stores pipeline behind PE.
            for s in range(ST):
                if s == 0:
                    tensor.wait_ge(s_l1, 16)    # xt k0,k1 + wp g0
                elif s == 1:
                    tensor.wait_ge(s_l3, 16)    # wp g1
                elif s == 2:
                    tensor.wait_ge(s_l4, 16)    # wp g2,g3
                elif s == 4:
                    tensor.wait_ge(s_l5, 16)    # wp g4,g5
                elif s == 6:
                    tensor.wait_ge(s_l6, 16)    # wp g6,g7
                for k in range(KT):
                    if s == 0 and k == 2:
                        tensor.wait_ge(s_l2, 16)    # xt k2,k3
                    inst = nc.tensor.matmul(
                        ps[:, s, :],
                        ld_sb[:, WP_COL[s] + k * 128:WP_COL[s] + (k + 1) * 128],
                        ld_sb[:, XT_COL[k]:XT_COL[k] + NH],
                        start=(k == 0),
                        stop=(k == KT - 1),
                    )
                    if k == KT - 1:
                        inst.then_inc(s_mm, 1)

        @block.vector
        def _(vector):
            if not with_clears:
                vector.memset(scr_sb[:], 0).then_inc(s_ws, 1)
            vector.wait_ge(s_cv, 16)  # cv loaded
            for s in range(ST - 1):
                vector.wait_ge(s_mm, s + 1)
                inst = nc.vector.tensor_scalar_add(
                    out_sb[:, s, :], ps[:, s, :], cv_sb[:, s:s + 1]
                )
                if s % 2 == 0:
                    inst.then_inc(s_dve_sync, 1)
                else:
                    inst.then_inc(s_dve_act, 1)
            # Group 7 in halves so each half's store can launch as soon as
            # its bytes are in SBUF.
            vector.wait_ge(s_mm, ST)
            nc.vector.tensor_scalar_add(
                out_sb[:, 7, 0:SH], ps[:, 7, 0:SH], cv_sb[:, 7:8]
            ).then_inc(s_dve_sync, 1)
            nc.vector.tensor_scalar_add(
                out_sb[:, 7, SH:NH], ps[:, 7, SH:NH], cv_sb[:, 7:8]
            ).then_inc(s_dve_act, 1)

    return nc


def _get_program():
    nc = _cache.get("nc")
    if nc is None:
        nc = _build_program()
        _cache["nc"] = nc
    return nc


def _prep_in_maps(x, idx, fbt, opt):
    bf = ml_dtypes.bfloat16
    in_maps = []
    for b in range(B):
        w = opt[idx[b]].reshape(F, D, O)                     # [F,D,O] f32
        wpack = w.transpose(1, 0, 2).reshape(KT, 128, ST, 128)  # [k,p,s,c]
        wp_host = np.ascontiguousarray(
            wpack.transpose(1, 2, 0, 3).reshape(128, KT * FO)
        ).astype(bf)                                         # [p, s*512+k*128+c]
        bias = fbt[idx[b]]                                   # [F,D]
        cvec = np.einsum("fd,fdo->fo", bias, w).reshape(FO).astype(np.float32)
        cv = np.ascontiguousarray(cvec.reshape(ST, 128).T)   # [128, ST]
        for h in range(2):
            xtT = x[b, h * NH:(h + 1) * NH, :].T             # [D, NH]
            xt_host = np.ascontiguousarray(
                xtT.reshape(KT, 128, NH).transpose(1, 0, 2).reshape(128, KT * NH)
            ).astype(bf)                                     # [128, k*NH+col]
            ldh = np.empty((128, LD), dtype=bf)
            for k in range(KT):
                ldh[:, XT_COL[k]:XT_COL[k] + NH] = xt_host[
                    :, k * NH:(k + 1) * NH
                ]
            for s in range(ST):
                ldh[:, WP_COL[s]:WP_COL[s] + 512] = wp_host[
                    :, s * 512:(s + 1) * 512
                ]
            in_maps.append({"ld": ldh, "cv": cv})
    return in_maps


def _assemble(results):
    out = np.empty((B, N, F, O), dtype=np.float32)
    for c in range(8):
        b, h = divmod(c, 2)
        y = np.asarray(results[c]["y"]).astype(np.float32)   # [FO, NH]
        out[b, h * NH:(h + 1) * NH] = y.reshape(F, O, NH).transpose(2, 0, 1)
    return out


def _run(x, idx, feature_bias_table, out_projection_table, **run_kwargs):
    from concourse.bass_utils import run_bass_kernel_spmd

    x = np.asarray(x, dtype=np.float32)
    idx = np.asarray(idx).astype(np.int64)
    fbt = np.asarray(feature_bias_table, dtype=np.float32)
    opt = np.asarray(out_projection_table, dtype=np.float32)

    nc = _get_program()
    in_maps = _prep_in_maps(x, idx, fbt, opt)
    res = run_bass_kernel_spmd(nc, in_maps, core_ids=list(range(8)), **run_kwargs)
    return _assemble(res.results), res


def kernel(x, idx, feature_bias_table, out_projection_table):
    out, _ = _run(x, idx, feature_bias_table, out_projection_table)
    return out


# revision 23
# speedup vs baseline: 1.1350x; 1.0007x over previous
"""Trainium2 Bass kernel for nn_ModalDecoder (embedding_lookup).

Reference computation:
    w  = out_projection_table[idx].reshape(B, F, D, O)      # [B,F,D,O]
    b  = feature_bias_table[idx]                            # [B,F,D]
    xb = x[:, :, None, :] + b[:, None, :, :]                # [B,N,F,D]
    out = einsum('bnfd,bfdo->bnfo', xb, w)                  # [B,N,F,O]

Factorization (avoids the 128MB [B,N,F,D] intermediate):
    out[b, n, f, :] = x[b, n, :] @ W[b, f] + (bias[b, f] @ W[b, f])
The bias term is a per-(b,f) length-O vector, broadcast over n; it is
precomputed on host and added during the PSUM->SBUF drain (per-partition
scalar add on DVE / ACT).

Sharding: 8 cores = 4 values of b x 2 halves of N. Per core:
    y[fo, n] = Wpack[d, fo].T @ xT[d, n] + cvec[fo]
with Wpack = [D, F*O] (host-gathered tables packed side by side), xT the
transposed x half, both bf16 (PSUM accumulates fp32). y is [F*O, N/2] fp16
(upcast on host; the bf16 matmul rounding dominates the error).

Schedule (v4), tuned to measured DMA physics (per-dma issue ~0.65us of
sequencer time; ring startup ~0.8us; stream ~390GB/s; completion receipt
~1.0us after last byte; ring FIFO):
  - All loads ride the sync HWDGE ring from ONE fused DRAM tensor laid out
    in load order [xt01|wp0 | xt23 | wp1 | wp23 | wp45 | wp67], as 6
    transfers; the first (384KB) delivers everything matmul 1 needs with a
    single completion sem, and every later chunk lands with >=0.4us slack
    before the PE consumes it.
  - PE warmup runway: 6x512-free + 7x128-free dummy matmuls keep the PE
    busy from block start (HAM clock-gate window accumulates; warm
    2.4GHz ~4.5us in) and hand over to real matmuls with <=107ns
    granularity. Any PE idle before warm would reset the HAM window
    (costs ~4us - measured), so the runway overshoots the expected gate
    by ~0.4us on purpose.
  - Drains on DVE (749ns/group < 864ns PE group cadence), stores
    alternate rings per group. Tail: group 7 drains in halves, DVE || ACT
    (ACT's activation table is preloaded at block start - first use
    otherwise costs a 1.28us ACT_TABLE_LOAD), stored as two 64KB
    transfers on both rings.
  - HW-variant store DMAs carry no completion semaphore: nothing waits on
    them and the NEFF epilogue's queue DRAIN retires them; dropping the
    sem skips the last-byte-confirmed receipt in the measured span. The
    CoreSim variant keeps sems for the race detector.

Per-core HBM traffic: 1.5MB loads + 1MB stores (memory-bound).
"""

import numpy as np
import ml_dtypes

B, N, D, O, F, V = 4, 1024, 512, 64, 16, 64
NH = N // 2            # 512 rows of x per core
FO = F * O             # 1024 packed output columns
KT = D // 128          # 4 contraction chunks
ST = FO // 128         # 8 output-partition chunks
SH = NH // 2           # half-group column split for the tail stores
LD = KT * NH + KT * FO  # 6144 fused load columns

# Fused load-buffer column offsets (load order).
XT_COL = {0: 0, 1: 1024, 2: 1536, 3: 2048}       # xt k-chunk -> col
WP_COL = {0: 512, 1: 2560, 2: 3072, 3: 3584,
          4: 4096, 5: 4608, 6: 5120, 7: 5632}    # wp s-group -> col
# Load transfers: (col_start, col_end); chunk j gets its own semaphore.
LD_CHUNKS = [(0, 1024), (1024, 1536), (1536, 2560), (2560, 3072),
             (3072, 4096), (4096, 5120), (5120, 6144)]
HH = 256               # group-7 PE split: h0 = 0:HH, h1 = HH:NH

_cache: dict = {}


def _build_program(with_clears=True):
    # with_clears=True is the real (HW) program. The False variant is for
    # CoreSim validation: it enables the race detector, memsets the warmup
    # scratch (CoreSim rejects reads of uninitialized SBUF; on HW the
    # warmup inputs are garbage by design and never observed), and adds
    # completion sems to the store DMAs (race-detector requirement; the HW
    # variant relies on the epilogue DRAIN instead).
    import concourse.bass as bass
    import concourse.mybir as mybir

    bf16 = mybir.dt.bfloat16
    f16 = mybir.dt.float16
    f32 = mybir.dt.float32

    nc = bass.Bass(
        "TRN2",
        target_bir_lowering=False,
        debug=False,
        num_devices=8,
        detect_race_conditions=not with_clears,
    )

    ld_d = nc.dram_tensor("ld", [128, LD], bf16, kind="ExternalInput")
    cv_d = nc.dram_tensor("cv", [128, ST], f32, kind="ExternalInput")
    y_d = nc.dram_tensor("y", [FO, NH], f16, kind="ExternalOutput")

    yv = y_d.ap().rearrange("(g p) n -> p g n", p=128)  # [128, ST, NH]

    with (
        nc.sbuf_tensor("ld_sb", [128, LD], bf16) as ld_sb,
        nc.sbuf_tensor("cv_sb", [128, ST], f32) as cv_sb,
        nc.sbuf_tensor("out_sb", [128, ST, NH], f16) as out_sb,
        nc.sbuf_tensor("scr_sb", [128, NH], bf16) as scr_sb,
        nc.psum_tensor([128, ST, NH], f32) as ps,
        nc.semaphore("s_l1") as s_l1,
        nc.semaphore("s_l2") as s_l2,
        nc.semaphore("s_l3") as s_l3,
        nc.semaphore("s_l4") as s_l4,
        nc.semaphore("s_l5") as s_l5,
        nc.semaphore("s_l6") as s_l6,
        nc.semaphore("s_l7") as s_l7,
        nc.semaphore("s_cv") as s_cv,
        nc.semaphore("s_ws") as s_ws,
        nc.semaphore("s_mm") as s_mm,
        nc.semaphore("s_dve_sync") as s_dve_sync,
        nc.semaphore("s_dve_act") as s_dve_act,
        nc.semaphore("s_st") as s_st,
        nc.Block() as block,
    ):
        ld_sems = [s_l1, s_l2, s_l3, s_l4, s_l5, s_l6, s_l7]

        def store(eng, dst, src):
            # HWDGE DMAs must carry a sem update (codegen requirement);
            # nothing waits on s_st -- the epilogue DRAIN handles retirement.
            eng.dma_start(dst, src).then_inc(s_st, 16)

        @block.sync
        def _(sync):
            # All loads on one ring, in PE-gating order; chunk j's sem
            # implies chunks <j landed (per-engine FIFO).
            for (c0, c1), sem in zip(LD_CHUNKS, ld_sems):
                sync.dma_start(ld_sb[:, c0:c1], ld_d.ap()[:, c0:c1]).then_inc(
                    sem, 16
                )
            # Even-group stores, then the first half of group 7.
            for j, s in enumerate((0, 2, 4, 6)):
                sync.wait_ge(s_dve_sync, j + 1)
                store(sync, yv[:, s, :], out_sb[:, s, :])
            sync.wait_ge(s_dve_sync, 5)
            store(sync, yv[:, 7, 0:HH], out_sb[:, 7, 0:HH])
            # No final completion wait: the framework epilogue's DRAIN
            # retires in-flight DMAs.

        @block.scalar
        def _(scalar):
            # cv primes this ring's DMA path; only DVE consumes it.
            # (No ACT compute anywhere: an activation instruction's
            # ACT_TABLE_LOAD DMA wedged the HW when issued concurrently
            # with this kernel's dynamic-DMA traffic.)
            scalar.dma_start(cv_sb[:], cv_d.ap()).then_inc(s_cv, 16)
            for j, s in enumerate((1, 3, 5)):
                scalar.wait_ge(s_dve_act, j + 1)
                store(scalar, yv[:, s, :], out_sb[:, s, :])
            # Tail: second half of group 7.
            scalar.wait_ge(s_dve_act, 4)
            store(scalar, yv[:, 7, HH:NH], out_sb[:, 7, HH:NH])

        @block.tensor
        def _(tensor):
            # Warmup runway: keeps the PE busy from block start until the
            # first load gate. Coarse then fine, so the handover to real
            # work wastes <=107ns. scr_sb is never written on HW.
            if not with_clears:
                tensor.wait_ge(s_ws, 1)
            for _ in range(6):
                nc.tensor.matmul(
                    ps[:, ST - 1, :],
                    scr_sb[:, :128],
                    scr_sb[:],
                    start=True,
                    stop=True,
                )
            for _ in range(7):
                nc.tensor.matmul(
                    ps[:, ST - 1, 0:128],
                    scr_sb[:, :128],
                    scr_sb[:, 0:128],
                    start=True,
                    stop=True,
                )
            # Group-serial accumulation: group s finishes after its own 4
            # matmuls, so DVE drains + stores pipeline behind PE.
            for s in range(ST):
                if s == 0:
                    tensor.wait_ge(s_l1, 16)    # xt k0 + wp g0
                elif s == 1:
                    tensor.wait_ge(s_l4, 16)    # wp g1
                elif s == 2:
                    tensor.wait_ge(s_l5, 16)    # wp g2,g3
                elif s == 4:
                    tensor.wait_ge(s_l6, 16)    # wp g4,g5
                elif s == 6:
                    tensor.wait_ge(s_l7, 16)    # wp g6,g7
                if s == ST - 1:
                    # Last group in column halves: h0's drain+store chain
                    # launches ~0.2us before the final (tiny) h1 matmuls
                    # retire, shortening the post-PE tail.
                    for c0, c1 in ((0, HH), (HH, NH)):
                        for k in range(KT):
                            inst = nc.tensor.matmul(
                                ps[:, s, c0:c1],
                                ld_sb[:, WP_COL[s] + k * 128:
                                      WP_COL[s] + (k + 1) * 128],
                                ld_sb[:, XT_COL[k] + c0:XT_COL[k] + c1],
                                start=(k == 0),
                                stop=(k == KT - 1),
                            )
                            if k == KT - 1:
                                inst.then_inc(s_mm, 1)
                    continue
                for k in range(KT):
                    if s == 0 and k == 1:
                        tensor.wait_ge(s_l2, 16)    # xt k1
                    elif s == 0 and k == 2:
                        tensor.wait_ge(s_l3, 16)    # xt k2,k3
                    inst = nc.tensor.matmul(
                        ps[:, s, :],
                        ld_sb[:, WP_COL[s] + k * 128:WP_COL[s] + (k + 1) * 128],
                        ld_sb[:, XT_COL[k]:XT_COL[k] + NH],
                        start=(k == 0),
                        stop=(k == KT - 1),
                    )
                    if k == KT - 1:
                        inst.then_inc(s_mm, 1)

        @block.vector
        def _(vector):
            if not with_clears:
                vector.memset(scr_sb[:], 0).then_inc(s_ws, 1)
            vector.wait_ge(s_cv, 16)  # cv loaded
            for s in range(ST - 1):
                vector.wait_ge(s_mm, s + 1)
                inst = nc.vector.tensor_scalar_add(
                    out_sb[:, s, :], ps[:, s, :], cv_sb[:, s:s + 1]
                )
                if s % 2 == 0:
                    inst.then_inc(s_dve_sync, 1)
                else:
                    inst.then_inc(s_dve_act, 1)
            # Group-7 halves: h0's drain starts while the PE finishes
            # h1's matmuls (the g7 PE split), so the tail chain overlaps.
            vector.wait_ge(s_mm, ST)
            nc.vector.tensor_scalar_add(
                out_sb[:, 7, 0:HH], ps[:, 7, 0:HH], cv_sb[:, 7:8]
            ).then_inc(s_dve_sync, 1)
            vector.wait_ge(s_mm, ST + 1)
            nc.vector.tensor_scalar_add(
                out_sb[:, 7, HH:NH], ps[:, 7, HH:NH], cv_sb[:, 7:8]
            ).then_inc(s_dve_act, 1)

    return nc


def _get_program():
    nc = _cache.get("nc")
    if nc is None:
        nc = _build_program()
        _cache["nc"] = nc
    return nc


def _prep_in_maps(x, idx, fbt, opt):
    bf = ml_dtypes.bfloat16
    in_maps = []
    for b in range(B):
        w = opt[idx[b]].reshape(F, D, O)                     # [F,D,O] f32
        wpack = w.transpose(1, 0, 2).reshape(KT, 128, ST, 128)  # [k,p,s,c]
        wp_host = np.ascontiguousarray(
            wpack.transpose(1, 2, 0, 3).reshape(128, KT * FO)
        ).astype(bf)                                         # [p, s*512+k*128+c]
        bias = fbt[idx[b]]                                   # [F,D]
        cvec = np.einsum("fd,fdo->fo", bias, w).reshape(FO).astype(np.float32)
        cv = np.ascontiguousarray(cvec.reshape(ST, 128).T)   # [128, ST]
        for h in range(2):
            xtT = x[b, h * NH:(h + 1) * NH, :].T             # [D, NH]
            xt_host = np.ascontiguousarray(
                xtT.reshape(KT, 128, NH).transpose(1, 0, 2).reshape(128, KT * NH)
            ).astype(bf)                                     # [128, k*NH+col]
            ldh = np.empty((128, LD), dtype=bf)
            for k in range(KT):
                ldh[:, XT_COL[k]:XT_COL[k] + NH] = xt_host[
                    :, k * NH:(k + 1) * NH
                ]
            for s in range(ST):
                ldh[:, WP_COL[s]:WP_COL[s] + 512] = wp_host[
                    :, s * 512:(s + 1) * 512
                ]
            in_maps.append({"ld": ldh, "cv": cv})
    return in_maps


def _assemble(results):
    out = np.empty((B, N, F, O), dtype=np.float32)
    for c in range(8):
        b, h = divmod(c, 2)
        y = np.asarray(results[c]["y"]).astype(np.float32)   # [FO, NH]
        out[b, h * NH:(h + 1) * NH] = y.reshape(F, O, NH).transpose(2, 0, 1)
    return out


def _run(x, idx, feature_bias_table, out_projection_table, **run_kwargs):
    from concourse.bass_utils import run_bass_kernel_spmd

    x = np.asarray(x, dtype=np.float32)
    idx = np.asarray(idx).astype(np.int64)
    fbt = np.asarray(feature_bias_table, dtype=np.float32)
    opt = np.asarray(out_projection_table, dtype=np.float32)

    nc = _get_program()
    in_maps = _prep_in_maps(x, idx, fbt, opt)
    res = run_bass_kernel_spmd(nc, in_maps, core_ids=list(range(8)), **run_kwargs)
    return _assemble(res.results), res


def kernel(x, idx, feature_bias_table, out_projection_table):
    out, _ = _run(x, idx, feature_bias_table, out_projection_table)
    return out


# revision 25
# speedup vs baseline: 1.1366x; 1.0014x over previous
"""Trainium2 Bass kernel for nn_ModalDecoder (embedding_lookup).

Reference computation:
    w  = out_projection_table[idx].reshape(B, F, D, O)      # [B,F,D,O]
    b  = feature_bias_table[idx]                            # [B,F,D]
    xb = x[:, :, None, :] + b[:, None, :, :]                # [B,N,F,D]
    out = einsum('bnfd,bfdo->bnfo', xb, w)                  # [B,N,F,O]

Factorization (avoids the 128MB [B,N,F,D] intermediate):
    out[b, n, f, :] = x[b, n, :] @ W[b, f] + (bias[b, f] @ W[b, f])
The bias term is a per-(b,f) length-O vector, broadcast over n; it is
precomputed on host and added during the PSUM->SBUF drain (per-partition
scalar add on DVE / ACT).

Sharding: 8 cores = 4 values of b x 2 halves of N. Per core:
    y[fo, n] = Wpack[d, fo].T @ xT[d, n] + cvec[fo]
with Wpack = [D, F*O] (host-gathered tables packed side by side), xT the
transposed x half, both bf16 (PSUM accumulates fp32). y is [F*O, N/2] fp16
(upcast on host; the bf16 matmul rounding dominates the error).

Schedule (v4), tuned to measured DMA physics (per-dma issue ~0.65us of
sequencer time; ring startup ~0.8us; stream ~390GB/s; completion receipt
~1.0us after last byte; ring FIFO):
  - All loads ride the sync HWDGE ring from ONE fused DRAM tensor laid out
    in load order [xt01|wp0 | xt23 | wp1 | wp23 | wp45 | wp67], as 6
    transfers; the first (384KB) delivers everything matmul 1 needs with a
    single completion sem, and every later chunk lands with >=0.4us slack
    before the PE consumes it.
  - PE warmup runway: 6x512-free + 7x128-free dummy matmuls keep the PE
    busy from block start (HAM clock-gate window accumulates; warm
    2.4GHz ~4.5us in) and hand over to real matmuls with <=107ns
    granularity. Any PE idle before warm would reset the HAM window
    (costs ~4us - measured), so the runway overshoots the expected gate
    by ~0.4us on purpose.
  - Drains on DVE (749ns/group < 864ns PE group cadence), stores
    alternate rings per group. Tail: group 7 drains in halves, DVE || ACT
    (ACT's activation table is preloaded at block start - first use
    otherwise costs a 1.28us ACT_TABLE_LOAD), stored as two 64KB
    transfers on both rings.
  - HW-variant store DMAs carry no completion semaphore: nothing waits on
    them and the NEFF epilogue's queue DRAIN retires them; dropping the
    sem skips the last-byte-confirmed receipt in the measured span. The
    CoreSim variant keeps sems for the race detector.

Per-core HBM traffic: 1.5MB loads + 1MB stores (memory-bound).
"""

import numpy as np
import ml_dtypes

B, N, D, O, F, V = 4, 1024, 512, 64, 16, 64
NH = N // 2            # 512 rows of x per core
FO = F * O             # 1024 packed output columns
KT = D // 128          # 4 contraction chunks
ST = FO // 128         # 8 output-partition chunks
SH = NH // 2           # half-group column split for the tail stores
LD = KT * NH + KT * FO  # 6144 fused load columns

# Fused load-buffer column offsets (load order).
XT_COL = {0: 0, 1: 1024, 2: 1536, 3: 2048}       # xt k-chunk -> col
WP_COL = {0: 512, 1: 2560, 2: 3072, 3: 3584,
          4: 4096, 5: 4608, 6: 5120, 7: 5632}    # wp s-group -> col
# Load transfers: (col_start, col_end); chunk j gets its own semaphore.
LD_CHUNKS = [(0, 1024), (1024, 2048), (2048, 2560), (2560, 3072),
             (3072, 4096), (4096, 5120), (5120, 6144)]
HH = 256               # group-7 PE split: h0 = 0:HH, h1 = HH:NH

_cache: dict = {}


def _build_program(with_clears=True):
    # with_clears=True is the real (HW) program. The False variant is for
    # CoreSim validation: it enables the race detector, memsets the warmup
    # scratch (CoreSim rejects reads of uninitialized SBUF; on HW the
    # warmup inputs are garbage by design and never observed), and adds
    # completion sems to the store DMAs (race-detector requirement; the HW
    # variant relies on the epilogue DRAIN instead).
    import concourse.bass as bass
    import concourse.mybir as mybir

    bf16 = mybir.dt.bfloat16
    f16 = mybir.dt.float16
    f32 = mybir.dt.float32

    nc = bass.Bass(
        "TRN2",
        target_bir_lowering=False,
        debug=False,
        num_devices=8,
        detect_race_conditions=not with_clears,
    )

    ld_d = nc.dram_tensor("ld", [128, LD], bf16, kind="ExternalInput")
    cv_d = nc.dram_tensor("cv", [128, ST], f32, kind="ExternalInput")
    y_d = nc.dram_tensor("y", [FO, NH], f16, kind="ExternalOutput")

    yv = y_d.ap().rearrange("(g p) n -> p g n", p=128)  # [128, ST, NH]

    with (
        nc.sbuf_tensor("ld_sb", [128, LD], bf16) as ld_sb,
        nc.sbuf_tensor("cv_sb", [128, ST], f32) as cv_sb,
        nc.sbuf_tensor("out_sb", [128, ST, NH], f16) as out_sb,
        nc.sbuf_tensor("scr_sb", [128, NH], bf16) as scr_sb,
        nc.psum_tensor([128, ST, NH], f32) as ps,
        nc.semaphore("s_l1") as s_l1,
        nc.semaphore("s_l2") as s_l2,
        nc.semaphore("s_l3") as s_l3,
        nc.semaphore("s_l4") as s_l4,
        nc.semaphore("s_l5") as s_l5,
        nc.semaphore("s_l6") as s_l6,
        nc.semaphore("s_l7") as s_l7,
        nc.semaphore("s_cv") as s_cv,
        nc.semaphore("s_ws") as s_ws,
        nc.semaphore("s_mm") as s_mm,
        nc.semaphore("s_dve_sync") as s_dve_sync,
        nc.semaphore("s_dve_act") as s_dve_act,
        nc.semaphore("s_st") as s_st,
        nc.Block() as block,
    ):
        ld_sems = [s_l1, s_l2, s_l3, s_l4, s_l5, s_l6, s_l7]

        def store(eng, dst, src):
            # HWDGE DMAs must carry a sem update (codegen requirement);
            # nothing waits on s_st -- the epilogue DRAIN handles retirement.
            eng.dma_start(dst, src).then_inc(s_st, 16)

        @block.sync
        def _(sync):
            # All loads on one ring, in PE-gating order; chunk j's sem
            # implies chunks <j landed (per-engine FIFO).
            for (c0, c1), sem in zip(LD_CHUNKS, ld_sems):
                sync.dma_start(ld_sb[:, c0:c1], ld_d.ap()[:, c0:c1]).then_inc(
                    sem, 16
                )
            # Even-group stores, then the first half of group 7.
            for j, s in enumerate((0, 2, 4, 6)):
                sync.wait_ge(s_dve_sync, j + 1)
                store(sync, yv[:, s, :], out_sb[:, s, :])
            sync.wait_ge(s_dve_sync, 5)      # g7h0 drained
            store(sync, yv[:, 7, 0:HH], out_sb[:, 7, 0:HH])
            # No final completion wait: the framework epilogue's DRAIN
            # retires in-flight DMAs.

        @block.scalar
        def _(scalar):
            # cv primes this ring's DMA path; only DVE consumes it.
            # (No ACT compute anywhere: an activation instruction's
            # ACT_TABLE_LOAD DMA wedged the HW when issued concurrently
            # with this kernel's dynamic-DMA traffic.)
            scalar.dma_start(cv_sb[:], cv_d.ap()).then_inc(s_cv, 16)
            for j, s in enumerate((1, 3, 5)):
                scalar.wait_ge(s_dve_act, j + 1)
                store(scalar, yv[:, s, :], out_sb[:, s, :])
            # Tail: second half of group 7.
            scalar.wait_ge(s_dve_act, 4)
            store(scalar, yv[:, 7, HH:NH], out_sb[:, 7, HH:NH])

        @block.tensor
        def _(tensor):
            # Warmup runway: keeps the PE busy from block start until the
            # first load gate. Coarse then fine, so the handover to real
            # work wastes <=107ns. scr_sb is never written on HW.
            if not with_clears:
                tensor.wait_ge(s_ws, 1)
            for _ in range(6):
                nc.tensor.matmul(
                    ps[:, ST - 1, :],
                    scr_sb[:, :128],
                    scr_sb[:],
                    start=True,
                    stop=True,
                )
            for _ in range(7):
                nc.tensor.matmul(
                    ps[:, ST - 1, 0:128],
                    scr_sb[:, :128],
                    scr_sb[:, 0:128],
                    start=True,
                    stop=True,
                )
            # Group-serial accumulation: group s finishes after its own 4
            # matmuls, so DVE drains + stores pipeline behind PE.
            for s in range(ST):
                if s == 0:
                    tensor.wait_ge(s_l1, 16)    # xt k0 + wp g0
                elif s == 1:
                    tensor.wait_ge(s_l4, 16)    # wp g1
                elif s == 2:
                    tensor.wait_ge(s_l5, 16)    # wp g2,g3
                elif s == 4:
                    tensor.wait_ge(s_l6, 16)    # wp g4,g5
                elif s == 6:
                    tensor.wait_ge(s_l7, 16)    # wp g6,g7
                if s == ST - 1:
                    # Last group in column halves so the tail drain+store
                    # chain starts before the final matmuls retire.
                    for c0, c1 in ((0, HH), (HH, NH)):
                        for k in range(KT):
                            inst = nc.tensor.matmul(
                                ps[:, s, c0:c1],
                                ld_sb[:, WP_COL[s] + k * 128:
                                      WP_COL[s] + (k + 1) * 128],
                                ld_sb[:, XT_COL[k] + c0:XT_COL[k] + c1],
                                start=(k == 0),
                                stop=(k == KT - 1),
                            )
                            if k == KT - 1:
                                inst.then_inc(s_mm, 1)
                    continue
                for k in range(KT):
                    if s == 0 and k == 1:
                        tensor.wait_ge(s_l2, 16)    # xt k1,k2
                    elif s == 0 and k == 3:
                        tensor.wait_ge(s_l3, 16)    # xt k3
                    inst = nc.tensor.matmul(
                        ps[:, s, :],
                        ld_sb[:, WP_COL[s] + k * 128:WP_COL[s] + (k + 1) * 128],
                        ld_sb[:, XT_COL[k]:XT_COL[k] + NH],
                        start=(k == 0),
                        stop=(k == KT - 1),
                    )
                    if k == KT - 1:
                        inst.then_inc(s_mm, 1)

        @block.vector
        def _(vector):
            if not with_clears:
                vector.memset(scr_sb[:], 0).then_inc(s_ws, 1)
            vector.wait_ge(s_cv, 16)  # cv loaded
            for s in range(ST - 1):
                vector.wait_ge(s_mm, s + 1)
                inst = nc.vector.tensor_scalar_add(
                    out_sb[:, s, :], ps[:, s, :], cv_sb[:, s:s + 1]
                )
                if s % 2 == 0:
                    inst.then_inc(s_dve_sync, 1)
                else:
                    inst.then_inc(s_dve_act, 1)
            # Group-7 halves (s_mm: g7h0=8, g7h1=9).
            vector.wait_ge(s_mm, ST)
            nc.vector.tensor_scalar_add(
                out_sb[:, 7, 0:HH], ps[:, 7, 0:HH], cv_sb[:, 7:8]
            ).then_inc(s_dve_sync, 1)
            vector.wait_ge(s_mm, ST + 1)
            nc.vector.tensor_scalar_add(
                out_sb[:, 7, HH:NH], ps[:, 7, HH:NH], cv_sb[:, 7:8]
            ).then_inc(s_dve_act, 1)

    return nc


def _get_program():
    nc = _cache.get("nc")
    if nc is None:
        nc = _build_program()
        _cache["nc"] = nc
    return nc


def _prep_in_maps(x, idx, fbt, opt):
    bf = ml_dtypes.bfloat16
    in_maps = []
    for b in range(B):
        w = opt[idx[b]].reshape(F, D, O)                     # [F,D,O] f32
        wpack = w.transpose(1, 0, 2).reshape(KT, 128, ST, 128)  # [k,p,s,c]
        wp_host = np.ascontiguousarray(
            wpack.transpose(1, 2, 0, 3).reshape(128, KT * FO)
        ).astype(bf)                                         # [p, s*512+k*128+c]
        bias = fbt[idx[b]]                                   # [F,D]
        cvec = np.einsum("fd,fdo->fo", bias, w).reshape(FO).astype(np.float32)
        cv = np.ascontiguousarray(cvec.reshape(ST, 128).T)   # [128, ST]
        for h in range(2):
            xtT = x[b, h * NH:(h + 1) * NH, :].T             # [D, NH]
            xt_host = np.ascontiguousarray(
                xtT.reshape(KT, 128, NH).transpose(1, 0, 2).reshape(128, KT * NH)
            ).astype(bf)                                     # [128, k*NH+col]
            ldh = np.empty((128, LD), dtype=bf)
            for k in range(KT):
                ldh[:, XT_COL[k]:XT_COL[k] + NH] = xt_host[
                    :, k * NH:(k + 1) * NH
                ]
            for s in range(ST):
                ldh[:, WP_COL[s]:WP_COL[s] + 512] = wp_host[
                    :, s * 512:(s + 1) * 512
                ]
            in_maps.append({"ld": ldh, "cv": cv})
    return in_maps


def _assemble(results):
    out = np.empty((B, N, F, O), dtype=np.float32)
    for c in range(8):
        b, h = divmod(c, 2)
        y = np.asarray(results[c]["y"]).astype(np.float32)   # [FO, NH]
        out[b, h * NH:(h + 1) * NH] = y.reshape(F, O, NH).transpose(2, 0, 1)
    return out


def _run(x, idx, feature_bias_table, out_projection_table, **run_kwargs):
    from concourse.bass_utils import run_bass_kernel_spmd

    x = np.asarray(x, dtype=np.float32)
    idx = np.asarray(idx).astype(np.int64)
    fbt = np.asarray(feature_bias_table, dtype=np.float32)
    opt = np.asarray(out_projection_table, dtype=np.float32)

    nc = _get_program()
    in_maps = _prep_in_maps(x, idx, fbt, opt)
    res = run_bass_kernel_spmd(nc, in_maps, core_ids=list(range(8)), **run_kwargs)
    return _assemble(res.results), res


def kernel(x, idx, feature_bias_table, out_projection_table):
    out, _ = _run(x, idx, feature_bias_table, out_projection_table)
    return out


# revision 29
# speedup vs baseline: 1.1418x; 1.0046x over previous
"""Trainium2 Bass kernel for nn_ModalDecoder (embedding_lookup).

Reference computation:
    w  = out_projection_table[idx].reshape(B, F, D, O)      # [B,F,D,O]
    b  = feature_bias_table[idx]                            # [B,F,D]
    xb = x[:, :, None, :] + b[:, None, :, :]                # [B,N,F,D]
    out = einsum('bnfd,bfdo->bnfo', xb, w)                  # [B,N,F,O]

Factorization (avoids the 128MB [B,N,F,D] intermediate):
    out[b, n, f, :] = x[b, n, :] @ W[b, f] + (bias[b, f] @ W[b, f])
The bias term is a per-(b,f) length-O vector, broadcast over n; it is
precomputed on host and added during the PSUM->SBUF drain (per-partition
scalar add on DVE / ACT).

Sharding: 8 cores = 4 values of b x 2 halves of N. Per core:
    y[fo, n] = Wpack[d, fo].T @ xT[d, n] + cvec[fo]
with Wpack = [D, F*O] (host-gathered tables packed side by side), xT the
transposed x half, both bf16 (PSUM accumulates fp32). y is [F*O, N/2] fp16
(upcast on host; the bf16 matmul rounding dominates the error).

Schedule (v6b), tuned to measured DMA physics (per-dma issue ~0.65us of
sequencer time; first-transfer ring startup ~0.8us; stream ~390GB/s; a
DMA's completion SEMAPHORE becomes visible to a waiting engine only
~1.5us after its last byte lands, while engine-to-engine sems take
~40-150ns; ring FIFO orders transfers and their sem incs):
  - All loads ride the sync HWDGE ring from ONE fused DRAM tensor laid
    out in load order [xtk0|wp0][xtk1|xtk2][xtk3][wp1][wp23][wp45][wp67]
    as 7 transfers with one sem each. The first chunk (256KB) delivers
    everything matmul 1 needs ~3.3us after block start, and each later
    chunk's sem lands before the (cold-rate) PE reaches its gate - a
    chunk boundary that arrives late idles the PE and, before the HAM
    clock-gate warms, a >~1us idle resets the warm window (~4us lost).
  - PE warmup runway: 6x512-free + 7x128-free dummy matmuls keep the PE
    busy from block start (HAM warms 1.2->2.4GHz after ~2.5-4.5us of
    sustained activity) and hand over to real matmuls with <=107ns
    granularity.
  - Drains on DVE (749ns/group < 864ns PE group cadence), stores
    alternate rings per group. Tail: group 7's PE work is emitted as two
    256-column halves (s_mm 8 and 9) so h0's drain+store chain launches
    ~0.44us before the final matmuls retire; the halves are stored as
    two 64KB transfers on different rings.
  - The ACT and GPSIMD engines are compute-idle by necessity: GPSIMD
    cannot access PSUM (BIR verifier), and any ACT activation issued
    mid-kernel (its first use triggers an ACT_TABLE_LOAD DMA) wedged the
    device unrecoverably in three separate placements. DVE is the only
    PSUM-drain engine.
  - Store DMAs carry a dummy sem (HWDGE codegen requires sync info);
    nothing waits on it - the NEFF epilogue's queue DRAIN retires
    in-flight stores.

Per-core HBM traffic: 1.5MB loads + 1MB stores (memory-bound).
"""

import numpy as np
import ml_dtypes

B, N, D, O, F, V = 4, 1024, 512, 64, 16, 64
NH = N // 2            # 512 rows of x per core
FO = F * O             # 1024 packed output columns
KT = D // 128          # 4 contraction chunks
ST = FO // 128         # 8 output-partition chunks
SH = NH // 2           # half-group column split for the tail stores
LD = KT * NH + KT * FO  # 6144 fused load columns

# Fused load-buffer column offsets (load order).
XT_COL = {0: 0, 1: 1024, 2: 1536, 3: 2048}       # xt k-chunk -> col
WP_COL = {0: 512, 1: 2560, 2: 3072, 3: 3584,
          4: 4096, 5: 4608, 6: 5120, 7: 5632}    # wp s-group -> col
# Load transfers: (col_start, col_end); chunk j gets its own semaphore.
LD_CHUNKS = [(0, 1024), (1024, 2048), (2048, 2560), (2560, 3072),
             (3072, 4096), (4096, 5120), (5120, 6144)]
HH = 256               # group-7 PE split: h0 = 0:HH, h1 = HH:NH

_cache: dict = {}


def _build_program(with_clears=True):
    # with_clears=True is the real (HW) program. The False variant is for
    # CoreSim validation: it enables the race detector, memsets the warmup
    # scratch (CoreSim rejects reads of uninitialized SBUF; on HW the
    # warmup inputs are garbage by design and never observed), and adds
    # completion sems to the store DMAs (race-detector requirement; the HW
    # variant relies on the epilogue DRAIN instead).
    import concourse.bass as bass
    import concourse.mybir as mybir

    bf16 = mybir.dt.bfloat16
    f16 = mybir.dt.float16
    f32 = mybir.dt.float32

    nc = bass.Bass(
        "TRN2",
        target_bir_lowering=False,
        debug=False,
        num_devices=8,
        detect_race_conditions=not with_clears,
    )

    ld_d = nc.dram_tensor("ld", [128, LD], bf16, kind="ExternalInput")
    cv_d = nc.dram_tensor("cv", [128, ST], f32, kind="ExternalInput")
    y_d = nc.dram_tensor("y", [FO, NH], f16, kind="ExternalOutput")

    yv = y_d.ap().rearrange("(g p) n -> p g n", p=128)  # [128, ST, NH]

    with (
        nc.sbuf_tensor("ld_sb", [128, LD], bf16) as ld_sb,
        nc.sbuf_tensor("cv_sb", [128, ST], f32) as cv_sb,
        nc.sbuf_tensor("out_sb", [128, ST, NH], f16) as out_sb,
        nc.sbuf_tensor("scr_sb", [128, NH], bf16) as scr_sb,
        nc.psum_tensor([128, ST, NH], f32) as ps,
        nc.semaphore("s_l1") as s_l1,
        nc.semaphore("s_l2") as s_l2,
        nc.semaphore("s_l3") as s_l3,
        nc.semaphore("s_l4") as s_l4,
        nc.semaphore("s_l5") as s_l5,
        nc.semaphore("s_l6") as s_l6,
        nc.semaphore("s_l7") as s_l7,
        nc.semaphore("s_cv") as s_cv,
        nc.semaphore("s_ws") as s_ws,
        nc.semaphore("s_mm") as s_mm,
        nc.semaphore("s_dve_sync") as s_dve_sync,
        nc.semaphore("s_dve_act") as s_dve_act,
        nc.semaphore("s_st") as s_st,
        nc.Block() as block,
    ):
        ld_sems = [s_l1, s_l2, s_l3, s_l4, s_l5, s_l6, s_l7]

        def store(eng, dst, src):
            # HWDGE DMAs must carry a sem update (codegen requirement);
            # nothing waits on s_st -- the epilogue DRAIN handles retirement.
            eng.dma_start(dst, src).then_inc(s_st, 16)

        @block.sync
        def _(sync):
            # All loads on one ring, in PE-gating order; chunk j's sem
            # implies chunks <j landed (per-engine FIFO).
            for (c0, c1), sem in zip(LD_CHUNKS, ld_sems):
                sync.dma_start(ld_sb[:, c0:c1], ld_d.ap()[:, c0:c1]).then_inc(
                    sem, 16
                )
            # Even-group stores, then the first half of group 7.
            for j, s in enumerate((0, 2, 4, 6)):
                sync.wait_ge(s_dve_sync, j + 1)
                store(sync, yv[:, s, :], out_sb[:, s, :])
            sync.wait_ge(s_dve_sync, 5)      # g7h0 drained
            store(sync, yv[:, 7, 0:HH], out_sb[:, 7, 0:HH])
            # No final completion wait: the framework epilogue's DRAIN
            # retires in-flight DMAs.

        @block.scalar
        def _(scalar):
            # cv primes this ring's DMA path; only DVE consumes it.
            # (No ACT compute anywhere: an activation instruction's
            # ACT_TABLE_LOAD DMA wedged the HW when issued concurrently
            # with this kernel's dynamic-DMA traffic.)
            scalar.dma_start(cv_sb[:], cv_d.ap()).then_inc(s_cv, 16)
            for j, s in enumerate((1, 3, 5)):
                scalar.wait_ge(s_dve_act, j + 1)
                store(scalar, yv[:, s, :], out_sb[:, s, :])
            # Tail: second half of group 7.
            scalar.wait_ge(s_dve_act, 4)
            store(scalar, yv[:, 7, HH:NH], out_sb[:, 7, HH:NH])

        @block.tensor
        def _(tensor):
            # Warmup runway: keeps the PE busy from block start until the
            # first load gate. Coarse then fine, so the handover to real
            # work wastes <=107ns. scr_sb is never written on HW.
            if not with_clears:
                tensor.wait_ge(s_ws, 1)
            for _ in range(6):
                nc.tensor.matmul(
                    ps[:, ST - 1, :],
                    scr_sb[:, :128],
                    scr_sb[:],
                    start=True,
                    stop=True,
                )
            for _ in range(7):
                nc.tensor.matmul(
                    ps[:, ST - 1, 0:128],
                    scr_sb[:, :128],
                    scr_sb[:, 0:128],
                    start=True,
                    stop=True,
                )
            # Group-serial accumulation: group s finishes after its own 4
            # matmuls, so DVE drains + stores pipeline behind PE.
            for s in range(ST):
                if s == 0:
                    tensor.wait_ge(s_l1, 16)    # xt k0 + wp g0
                elif s == 1:
                    tensor.wait_ge(s_l4, 16)    # wp g1
                elif s == 2:
                    tensor.wait_ge(s_l5, 16)    # wp g2,g3
                elif s == 4:
                    tensor.wait_ge(s_l6, 16)    # wp g4,g5
                elif s == 6:
                    tensor.wait_ge(s_l7, 16)    # wp g6,g7
                if s == ST - 1:
                    # Last group in column halves so the tail drain+store
                    # chain starts before the final matmuls retire.
                    for c0, c1 in ((0, HH), (HH, NH)):
                        for k in range(KT):
                            inst = nc.tensor.matmul(
                                ps[:, s, c0:c1],
                                ld_sb[:, WP_COL[s] + k * 128:
                                      WP_COL[s] + (k + 1) * 128],
                                ld_sb[:, XT_COL[k] + c0:XT_COL[k] + c1],
                                start=(k == 0),
                                stop=(k == KT - 1),
                            )
                            if k == KT - 1:
                                inst.then_inc(s_mm, 1)
                    continue
                for k in range(KT):
                    if s == 0 and k == 1:
                        tensor.wait_ge(s_l2, 16)    # xt k1,k2
                    elif s == 0 and k == 3:
                        tensor.wait_ge(s_l3, 16)    # xt k3
                    inst = nc.tensor.matmul(
                        ps[:, s, :],
                        ld_sb[:, WP_COL[s] + k * 128:WP_COL[s] + (k + 1) * 128],
                        ld_sb[:, XT_COL[k]:XT_COL[k] + NH],
                        start=(k == 0),
                        stop=(k == KT - 1),
                    )
                    if k == KT - 1:
                        inst.then_inc(s_mm, 1)

        @block.vector
        def _(vector):
            if not with_clears:
                vector.memset(scr_sb[:], 0).then_inc(s_ws, 1)
            vector.wait_ge(s_cv, 16)  # cv loaded
            for s in range(ST - 1):
                vector.wait_ge(s_mm, s + 1)
                inst = nc.vector.tensor_scalar_add(
                    out_sb[:, s, :], ps[:, s, :], cv_sb[:, s:s + 1]
                )
                if s % 2 == 0:
                    inst.then_inc(s_dve_sync, 1)
                else:
                    inst.then_inc(s_dve_act, 1)
            # Group-7 halves (s_mm: g7h0=8, g7h1=9).
            vector.wait_ge(s_mm, ST)
            nc.vector.tensor_scalar_add(
                out_sb[:, 7, 0:HH], ps[:, 7, 0:HH], cv_sb[:, 7:8]
            ).then_inc(s_dve_sync, 1)
            vector.wait_ge(s_mm, ST + 1)
            nc.vector.tensor_scalar_add(
                out_sb[:, 7, HH:NH], ps[:, 7, HH:NH], cv_sb[:, 7:8]
            ).then_inc(s_dve_act, 1)

    return nc


def _get_program():
    nc = _cache.get("nc")
    if nc is None:
        nc = _build_program()
        _cache["nc"] = nc
    return nc


def _prep_in_maps(x, idx, fbt, opt):
    bf = ml_dtypes.bfloat16
    in_maps = []
    for b in range(B):
        w = opt[idx[b]].reshape(F, D, O)                     # [F,D,O] f32
        wpack = w.transpose(1, 0, 2).reshape(KT, 128, ST, 128)  # [k,p,s,c]
        wp_host = np.ascontiguousarray(
            wpack.transpose(1, 2, 0, 3).reshape(128, KT * FO)
        ).astype(bf)                                         # [p, s*512+k*128+c]
        bias = fbt[idx[b]]                                   # [F,D]
        cvec = np.einsum("fd,fdo->fo", bias, w).reshape(FO).astype(np.float32)
        cv = np.ascontiguousarray(cvec.reshape(ST, 128).T)   # [128, ST]
        for h in range(2):
            xtT = x[b, h * NH:(h + 1) * NH, :].T             # [D, NH]
            xt_host = np.ascontiguousarray(
                xtT.reshape(KT, 128, NH).transpose(1, 0, 2).reshape(128, KT * NH)
            ).astype(bf)                                     # [128, k*NH+col]
            ldh = np.empty((128, LD), dtype=bf)
            for k in range(KT):
                ldh[:, XT_COL[k]:XT_COL[k] + NH] = xt_host[
                    :, k * NH:(k + 1) * NH
                ]
            for s in range(ST):
                ldh[:, WP_COL[s]:WP_COL[s] + 512] = wp_host[
                    :, s * 512:(s + 1) * 512
                ]
            in_maps.append({"ld": ldh, "cv": cv})
    return in_maps


def _assemble(results):
    out = np.empty((B, N, F, O), dtype=np.float32)
    for c in range(8):
        b, h = divmod(c, 2)
        y = np.asarray(results[c]["y"]).astype(np.float32)   # [FO, NH]
        out[b, h * NH:(h + 1) * NH] = y.reshape(F, O, NH).transpose(2, 0, 1)
    return out


def _run(x, idx, feature_bias_table, out_projection_table, **run_kwargs):
    from concourse.bass_utils import run_bass_kernel_spmd

    x = np.asarray(x, dtype=np.float32)
    idx = np.asarray(idx).astype(np.int64)
    fbt = np.asarray(feature_bias_table, dtype=np.float32)
    opt = np.asarray(out_projection_table, dtype=np.float32)

    nc = _get_program()
    in_maps = _prep_in_maps(x, idx, fbt, opt)
    res = run_bass_kernel_spmd(nc, in_maps, core_ids=list(range(8)), **run_kwargs)
    return _assemble(res.results), res


def kernel(x, idx, feature_bias_table, out_projection_table):
    out, _ = _run(x, idx, feature_bias_table, out_projection_table)
    return out
